# revision 28
# baseline (speedup 1.0000x reference)
"""Trainium2 Bass kernel for nn_AttentionBlock (pre-LN causal attention + SiLU MLP).

8-core SPMD strategy (data-parallel over batch x sequence-parallel over rows):
  - core c handles sample b = c // NPOS, position g = c % NPOS
  - the L rows of a sample are split into NBLK blocks of BS rows; each core owns
    NBPC blocks, paired to balance causal-attention cost (host-chosen pairing)
  - every core computes LN1 + K^T/V for the full sample (replicated), q/proj/MLP
    only for its own rows.
  - the program is branch-free and identical on every core: per-slot attention
    trip counts are the max over cores, and all per-core differences (which
    rows, causal masking) enter via input data (xT_own, selb, attnmask).

All matmul layouts are "transposed" (feature dim on partitions) so no on-device
transposes are needed anywhere; the host feeds x pre-transposed and re-assembles
the transposed output.

Engine balance: PE does all matmuls; ACT does exp/silu/rsqrt + PSUM->SBUF
copies; Pool (GpSimd) does squares, partition broadcasts and attention mask
multiplies; DVE keeps the remaining elementwise.
"""
import math
from contextlib import ExitStack
from dataclasses import dataclass

import ml_dtypes
import numpy as np

import concourse.bass as bass
import concourse.mybir as mybir
import concourse.tile as tile
from concourse import bacc
from concourse.bass import ds, ts
from concourse.bass_utils import run_bass_kernel_spmd

F32 = mybir.dt.float32
BF16 = mybir.dt.bfloat16
AF = mybir.ActivationFunctionType
ALU = mybir.AluOpType
BF16NP = ml_dtypes.bfloat16


@dataclass
class Cfg:
    B: int = 2
    L: int = 2048
    E: int = 768
    H: int = 12
    D: int = 64
    FF: int = 3072
    BS: int = 256          # query block rows
    n_cores: int = 8
    eps: float = 1e-6

    @property
    def NPOS(self):
        return self.n_cores // self.B

    @property
    def NBLK(self):
        return self.L // self.BS

    @property
    def NBPC(self):
        return self.NBLK // self.NPOS   # blocks per core

    @property
    def R(self):
        return self.NBPC * self.BS      # own rows per core

    @property
    def EC(self):
        return self.E // 128

    @property
    def FC(self):
        return self.FF // 128

    @property
    def LC(self):
        return self.L // 128

    @property
    def HC(self):
        return self.H // 2              # head-pair chunks (= EC since E = H*D, D=64)


def plan_blocks(cfg: Cfg, mask_lens):
    """Choose jmax (number of attention-active blocks) and block pairing."""
    mmax = int(max(int(m) for m in mask_lens))
    mmax = max(1, min(cfg.L, mmax))
    jmax = (mmax + cfg.BS - 1) // cfg.BS          # blocks [0, jmax) need causal attn
    def cost(j):
        return (j + 1) if j < jmax else 0
    order = sorted(range(cfg.NBLK), key=lambda j: -cost(j))
    pairs = []
    for g in range(cfg.NPOS):
        blocks = []
        for s in range(cfg.NBPC):
            # snake over sorted order: pair heavy with light
            idx = g if s % 2 == 0 else (cfg.NBLK - 1 - g)
            blocks.append(order[idx])
        pairs.append(tuple(blocks))
    return pairs, jmax


def kc_of(cfg: Cfg, j, jmax):
    """number of 128-wide key chunks block j attends to (0 if mask-free)."""
    if j >= jmax:
        return 0
    return (j + 1) * cfg.BS // 128


def slot_kcs(cfg: Cfg, pairs, jmax):
    """Uniform (branch-free) per-slot key-chunk trip counts: the max over
    cores. Cores whose block needs fewer chunks mask the excess to zero via
    the per-core attnmask input; fully-masked blocks compute garbage that the
    sel blend discards (their denominators stay positive: plain causal
    masks are supplied for every assigned block, active or not)."""
    return [max(kc_of(cfg, pairs[g][s], jmax) for g in range(len(pairs)))
            for s in range(cfg.NBPC)]


# packed small-consts layout (columns of the [128, CW] f32 "cst" input):
#   bq [0:EC)  bk [EC:2EC)  bp [2EC:3EC)  bout [3EC:4EC)
#   bfc [4EC : 4EC+FC)   ln (g1,b1,g2,b2) [4EC+FC : 4EC+FC+4EC)
def cst_width(cfg: Cfg):
    return 8 * cfg.EC + cfg.FC


# ----------------------------------------------------------------------------
# program builder
# ----------------------------------------------------------------------------

def build_program(cfg: Cfg, pairs, jmax, flags, bake_g=None, stage_limit=99, repeat=1,
                  loop_n=1, ablate=()):
    """flags: dict with bools: bq, bk, bv, bproj, bfc, bout, ln1aff, ln2aff"""
    E, L, H, D2, FF, BS, R = cfg.E, cfg.L, cfg.H, cfg.D, cfg.FF, cfg.BS, cfg.R
    EC, FC, LC, HC, NBPC = cfg.EC, cfg.FC, cfg.LC, cfg.HC, cfg.NBPC
    KEYS = jmax * BS
    KC = KEYS // 128
    KS = slot_kcs(cfg, pairs, jmax)      # uniform per-slot trip counts
    NK = sum(KS)
    KTC = max(KS) if KS else 0           # key chunks to compute kT for
    qscale = 1.0 / math.sqrt(cfg.D)
    CW = cst_width(cfg)
    WFCG = 6                             # wfc chunks per streamed DMA

    nc = bacc.Bacc(num_devices=cfg.n_cores)

    # ---- dram I/O ----
    d_xTf = nc.dram_tensor("xT_full", [128, EC * L], BF16, kind="ExternalInput")
    d_xTo = nc.dram_tensor("xT_own", [128, EC * R], F32, kind="ExternalInput")
    d_wq = nc.dram_tensor("wq", [128, EC * E], BF16, kind="ExternalInput")
    d_wk = nc.dram_tensor("wk", [128, EC * E], BF16, kind="ExternalInput")
    d_wv = nc.dram_tensor("wv", [128, EC * E], BF16, kind="ExternalInput")
    d_wp = nc.dram_tensor("wproj", [128, EC * E], BF16, kind="ExternalInput")
    d_wfc = nc.dram_tensor("wfc", [FC, 128, EC * 128], BF16, kind="ExternalInput")
    d_wout = nc.dram_tensor("wout", [FC, 128, EC * 128], BF16, kind="ExternalInput")
    d_cst = nc.dram_tensor("cst", [128, CW], F32, kind="ExternalInput")
    d_bv = nc.dram_tensor("bv", [1, E], BF16, kind="ExternalInput")
    d_selb = nc.dram_tensor("selb", [128, R], BF16, kind="ExternalInput")
    d_amask = nc.dram_tensor("attnmask", [NK, 128, BS], BF16, kind="ExternalInput")
    d_out = nc.dram_tensor("outT", [128, EC * R], F32, kind="ExternalOutput")

    with tile.TileContext(nc) as tc, ExitStack() as st:
        # ------- L0: persistent pools -------
        cpool = st.enter_context(tc.tile_pool(name="consts", bufs=1))

        wq_s = cpool.tile([128, EC, E], BF16)
        wk_s = cpool.tile([128, EC, E], BF16)
        wv_s = cpool.tile([128, EC, E], BF16)
        wp_s = cpool.tile([128, EC, E], BF16)
        xo_s = cpool.tile([128, EC, R], F32)
        cst_s = cpool.tile([128, CW], F32)
        bv_s = cpool.tile([1, E], BF16)
        selb_s = cpool.tile([128, R], BF16)
        amask_s = cpool.tile([128, NK, BS], BF16)
        ones_col = cpool.tile([128, 1], BF16)
        ones_colf = cpool.tile([128, 1], F32)
        ones_row = cpool.tile([1, 128], BF16)
        ones_11 = cpool.tile([1, 1], BF16)
        eps_11 = cpool.tile([1, 1], F32)
        yT = cpool.tile([128, HC, R], BF16)
        vbarT = cpool.tile([128, EC, 1], F32)
        vrow = cpool.tile([1, E], BF16)

        # packed const column views
        bq_c = cst_s[:, 0:EC]
        bk_c = cst_s[:, EC:2 * EC]
        bp_c = cst_s[:, 2 * EC:3 * EC]
        bout_c = cst_s[:, 3 * EC:4 * EC]
        bfc_c = cst_s[:, 4 * EC:4 * EC + FC]
        ln_c = cst_s[:, 4 * EC + FC:]      # [128, 4*EC] (g1,b1,g2,b2)

        def emit_body(ri):
          with tc.tile_pool(name=f"gpsum{ri}", bufs=2, space="PSUM") as gpsum:
            nc.sync.dma_start(xo_s[:], d_xTo.rearrange("p (c n) -> p c n", c=EC))
            nc.sync.dma_start(wq_s[:], d_wq.rearrange("p (c n) -> p c n", c=EC))
            nc.sync.dma_start(wk_s[:], d_wk.rearrange("p (c n) -> p c n", c=EC))
            nc.sync.dma_start(wv_s[:], d_wv.rearrange("p (c n) -> p c n", c=EC))
            nc.sync.dma_start(wp_s[:], d_wp.rearrange("p (c n) -> p c n", c=EC))
            nc.sync.dma_start(cst_s[:], d_cst[:])
            nc.sync.dma_start(bv_s[:], d_bv[:])
            nc.sync.dma_start(selb_s[:], d_selb[:])
            if "nomaskdma" in ablate:
                nc.vector.memset(amask_s[:], 1.0)
            else:
                nc.sync.dma_start(amask_s[:], d_amask.rearrange("k p q -> p k q"))
            nc.vector.memset(ones_col[:], 1.0)
            nc.vector.memset(ones_colf[:], 1.0)
            nc.vector.memset(ones_row[:], 1.0)
            nc.vector.memset(ones_11[:], 1.0)
            nc.vector.memset(eps_11[:], cfg.eps)
            nc.vector.memset(yT[:], 0.0)

            # ============================================================
            # layernorm helpers (transposed layout: tokens on the free axis)
            # stats per 512-column group -> broadcast a/b rows -> apply
            # ============================================================
            def ln_stats_group(pool, statp, abpool, x_bf, cg0, w, tag):
                ps_su = statp.tile([1, 512], F32, tag="su", name=f"su{tag}{cg0}")
                ps_sq = statp.tile([1, 512], F32, tag="sq", name=f"sq{tag}{cg0}")
                ones_in = ones_colf if x_bf.dtype == F32 else ones_col
                for c in range(EC):
                    nc.tensor.matmul(ps_su[:, :w], ones_in[:], x_bf[:, c, cg0:cg0 + w],
                                     start=(c == 0), stop=(c == EC - 1))
                for c in range(EC):
                    sq = pool.tile([128, 512], BF16, tag="lnsq", name=f"sq{tag}{cg0}{c}")
                    nc.vector.tensor_tensor(sq[:, :w], x_bf[:, c, cg0:cg0 + w],
                                            x_bf[:, c, cg0:cg0 + w], ALU.mult)
                    nc.tensor.matmul(ps_sq[:, :w], ones_col[:], sq[:, :w],
                                     start=(c == 0), stop=(c == EC - 1))
                # mu = su/E ; var = sq/E - mu^2 ; a = rsqrt(var+eps) ; b = -mu*a
                mu = pool.tile([1, 512], F32, tag="lnmu", name=f"mu{tag}{cg0}")
                nc.vector.tensor_scalar_mul(mu[:, :w], ps_su[:, :w], 1.0 / E)
                m2 = pool.tile([1, 512], F32, tag="lnm2", name=f"m2{tag}{cg0}")
                nc.vector.tensor_tensor(m2[:, :w], mu[:, :w], mu[:, :w], ALU.mult)
                va = pool.tile([1, 512], F32, tag="lnva", name=f"va{tag}{cg0}")
                nc.vector.scalar_tensor_tensor(va[:, :w], ps_sq[:, :w], 1.0 / E,
                                               m2[:, :w], ALU.mult, ALU.subtract)
                sd = pool.tile([1, 512], F32, tag="lnsd", name=f"sd{tag}{cg0}")
                nc.scalar.activation(sd[:, :w], va[:, :w], AF.Sqrt, bias=eps_11[:])
                arow = pool.tile([1, 512], BF16, tag="lnar", name=f"ar{tag}{cg0}")
                with nc.allow_low_precision(reason="1/std rounded to bf16 as before"):
                    nc.vector.reciprocal(arow[:, :w], sd[:, :w])
                brow = pool.tile([1, 512], BF16, tag="lnbr", name=f"br{tag}{cg0}")
                nc.vector.scalar_tensor_tensor(brow[:, :w], mu[:, :w], -1.0,
                                               arow[:, :w], ALU.mult, ALU.mult)
                ab = abpool.tile([128, 2, 512], BF16, tag="lnab", name=f"ab{tag}{cg0}")
                nc.gpsimd.partition_broadcast(ab[:, 0, :w], arow[:, :w])
                nc.gpsimd.partition_broadcast(ab[:, 1, :w], brow[:, :w])
                return ab

            def ln_apply_group(x_bf, zT_out, cg0, w, ab, gb_idx):
                gi, bi = gb_idx
                affine = flags["ln1aff"] if gb_idx == (0, 1) else flags["ln2aff"]
                for c in range(EC):
                    nc.vector.tensor_tensor(zT_out[:, c, cg0:cg0 + w],
                                            x_bf[:, c, cg0:cg0 + w], ab[:, 0, :w], ALU.mult)
                    nc.vector.tensor_tensor(zT_out[:, c, cg0:cg0 + w],
                                            zT_out[:, c, cg0:cg0 + w], ab[:, 1, :w], ALU.add)
                    if affine:
                        nc.vector.tensor_scalar(zT_out[:, c, cg0:cg0 + w],
                                                zT_out[:, c, cg0:cg0 + w],
                                                ln_c[:, gi * EC + c:gi * EC + c + 1],
                                                ln_c[:, bi * EC + c:bi * EC + c + 1],
                                                ALU.mult, ALU.add)

            # ------- L2: sample-wide tensors (die after attention) -------
            with tc.tile_pool(name="l2", bufs=1) as l2:
                zT = l2.tile([128, EC, L], BF16, tag="zT", name="zT")
                xf_ap = d_xTf.rearrange("p (c n) -> p c n", c=EC)
                for cg0 in range(0, L, 512):
                    nc.sync.dma_start(zT[:, :, cg0:cg0 + 512],
                                      xf_ap[:, :, cg0:cg0 + 512])
                zqT = l2.tile([128, EC, R], BF16, tag="zqT", name="zqT")
                qTs = l2.tile([128, HC, R], BF16, tag="qTs", name="qTs")
                kTs = l2.tile([128, HC, KEYS], BF16, tag="kTs", name="kTs")
                Vs = l2.tile([128, KC, H, 65], BF16, tag="Vs", name="Vs")
                nc.vector.memset(Vs[:, :, :, 64:65], 1.0)
                if "noqkv" in ablate:
                    nc.vector.memset(qTs[:], 0.001)
                    nc.vector.memset(kTs[:], 0.001)
                    nc.vector.memset(Vs[:, :, :, 0:64], 0.001)

                def emit_kT_group(cg0, w):
                    """kT output for key columns [cg0, cg0+w) (all EC head chunks)."""
                    for m in range(EC):
                        ps = gpsum.tile([128, 512], F32, tag="gp", name=f"psk{m}{cg0}")
                        for c in range(EC):
                            nc.tensor.matmul(ps[:, :w], wk_s[:, c, ts(m, 128)],
                                             zT[:, c, cg0:cg0 + w],
                                             start=(c == 0), stop=(c == EC - 1))
                        if flags["bk"]:
                            nc.scalar.activation(kTs[:, m, cg0:cg0 + w], ps[:, :w],
                                                 AF.Identity, bias=bk_c[:, m:m + 1])
                        else:
                            nc.scalar.copy(kTs[:, m, cg0:cg0 + w], ps[:, :w])

                def emit_V_chunk(r):
                    """V rows for key chunk r: [128(keyrow), H, 64] + ones col."""
                    for n0 in range(0, E, 512):
                        w = min(512, E - n0)
                        ps = gpsum.tile([128, 512], F32, tag="gp", name=f"psv{r}{n0}")
                        for c in range(EC):
                            nc.tensor.matmul(ps[:, :w], zT[:, c, ts(r, 128)],
                                             wv_s[:, c, n0:n0 + w],
                                             start=(c == 0),
                                             stop=(c == EC - 1 and not flags["bv"]))
                        if flags["bv"]:
                            nc.tensor.matmul(ps[:, :w], ones_row[:], bv_s[:, n0:n0 + w],
                                             start=False, stop=True)
                        h0 = n0 // 64
                        nh = w // 64
                        nc.scalar.copy(
                            Vs[:, r, h0:h0 + nh, 0:64],
                            ps[:, :w].rearrange("p (h d) -> p h d", d=64))

                # ------- LN1 over full L, interleaved with kT/V per group ----
                do_ln1 = stage_limit >= 1 and "noln1" not in ablate
                do_qkv = stage_limit >= 2 and "noqkv" not in ablate
                with tc.tile_pool(name="l3", bufs=2) as l3, \
                     tc.tile_pool(name="lnab1", bufs=2) as lnab1, \
                     tc.tile_pool(name=f"statp{ri}", bufs=2, space="PSUM") as statp:
                    # own-row LN (straight off the f32 xo input) + q projection:
                    # depends only on the small xo DMA, so it runs while zT loads
                    if do_ln1:
                        ab = ln_stats_group(l3, statp, lnab1, xo_s, 0, R, "o")
                        ln_apply_group(xo_s, zqT, 0, R, ab, (0, 1))
                    else:
                        nc.vector.tensor_copy(zqT[:], xo_s[:])
                    for m in range(EC if do_qkv else 0):
                        ps = gpsum.tile([128, 512], F32, tag="gp", name=f"psq{m}")
                        for c in range(EC):
                            nc.tensor.matmul(ps[:, :R], wq_s[:, c, ts(m, 128)], zqT[:, c, :],
                                             start=(c == 0), stop=(c == EC - 1))
                        # host pre-scales bq by qscale so Identity(in*qscale + bq') works
                        if flags["bq"]:
                            nc.scalar.activation(qTs[:, m, :], ps[:, :R], AF.Identity,
                                                 bias=bq_c[:, m:m + 1], scale=qscale)
                        else:
                            nc.scalar.activation(qTs[:, m, :], ps[:, :R], AF.Copy,
                                                 scale=qscale)
                    for cg0 in range(0, L, 512):
                        if do_ln1:
                            ab = ln_stats_group(l3, statp, lnab1, zT, cg0, 512, "f")
                            ln_apply_group(zT, zT, cg0, 512, ab, (0, 1))
                        if do_qkv and cg0 < KTC * 128:
                            emit_kT_group(cg0, min(512, KTC * 128 - cg0))
                        if do_qkv:
                            for r in range(cg0 // 128, min(KC, cg0 // 128 + 4)):
                                emit_V_chunk(r)

                # vbar = mean over all L rows of V, per head -> vbarT [128, EC, 1]
                if "novbar" in ablate:
                    nc.vector.memset(vbarT[:], 0.001)
                do_vbar = stage_limit >= 4 and "novbar" not in ablate
                HG = 512 // 65            # heads per vbar psum group
                for h0 in range(0, H if do_vbar else 0, HG):
                    nh = min(HG, H - h0)
                    ps = gpsum.tile([1, 512], F32, tag="gp", name=f"vb{h0}")
                    for r in range(KC):
                        nc.tensor.matmul(ps[:, :nh * 65], ones_col[:],
                                         Vs[:, r, h0:h0 + nh, :],
                                         start=(r == 0), stop=(r == KC - 1))
                    nc.vector.tensor_scalar_mul(
                        vrow[:, h0 * 64:(h0 + nh) * 64].rearrange("p (h d) -> p h d", d=64),
                        ps[:, :nh * 65].rearrange("p (h c) -> p h c", c=65)[:, :, 0:64],
                        1.0 / L)
                if KEYS < L and do_vbar:
                    # tail rows [KEYS, L): vbar += (sum of z rows) @ Wv / L
                    zsum = l2.tile([128, EC, 1], F32, tag="zsum", name="zsum")
                    for c in range(EC):
                        nc.vector.tensor_reduce(zsum[:, c, :], zT[:, c, KEYS:L],
                                                mybir.AxisListType.X, ALU.add)
                    zsumb = l2.tile([128, EC, 1], BF16, tag="zsumb", name="zsumb")
                    nc.vector.tensor_scalar_mul(zsumb[:], zsum[:], 1.0 / L)
                for m in range(EC if do_vbar else 0):
                    ps = gpsum.tile([128, 512], F32, tag="gp", name=f"vbt{m}")
                    nc.tensor.matmul(ps[:, 0:1], vrow[:, ts(m, 128)], ones_11[:],
                                     start=True, stop=(KEYS >= L))
                    if KEYS < L:
                        for c in range(EC):
                            nc.tensor.matmul(ps[:, 0:1], wv_s[:, c, ts(m, 128)],
                                             zsumb[:, c, :],
                                             start=False, stop=(c == EC - 1))
                    nc.vector.tensor_copy(vbarT[:, m, :], ps[:, 0:1])
                if KEYS < L and flags["bv"] and do_vbar:
                    # tail bias: vbar += (L-KEYS)/L * bv  (per hd on partitions)
                    bvt = l2.tile([128, EC, 1], BF16, tag="bvt", name="bvt")
                    nc.sync.dma_start(bvt[:], d_bv.rearrange("o (c p) -> p c o", p=128))
                    nc.vector.tensor_scalar(bvt[:], bvt[:], float(L - KEYS) / L, None,
                                            ALU.mult)
                    nc.vector.tensor_tensor(vbarT[:], vbarT[:], bvt[:], ALU.add)

                # ------- attention (branch-free, uniform trip counts) -------
                with (
                    tc.tile_pool(name="att", bufs=3) as att,
                    tc.tile_pool(name="spsum", bufs=2, space="PSUM") as spsum,
                    tc.tile_pool(name="ypsum", bufs=2, space="PSUM") as ypsum,
                ):
                    if stage_limit >= 3 and "noatt" not in ablate:
                        for slot in range(NBPC):
                            kc = KS[slot]
                            if kc == 0:
                                continue
                            base = sum(KS[:slot])
                            qsl = ds(slot * BS, BS)
                            for hp in range(HC):
                                ps_ys = []
                                for h01 in (0, 1):
                                    ps_y = ypsum.tile([65, BS], F32, tag="y",
                                                      name=f"y{slot}{hp}{h01}")
                                    ps_ys.append(ps_y)
                                kdone = 0
                                while kdone < kc:
                                    gsz = min(4, kc - kdone)
                                    for h01 in (0, 1):
                                        h = 2 * hp + h01
                                        pb = h01 * 64
                                        ps_s = spsum.tile([128, 4, BS], F32, tag="s",
                                                          name=f"s{slot}{hp}{h01}{kdone}")
                                        for i in range(gsz):
                                            ki = kdone + i
                                            nc.tensor.matmul(
                                                ps_s[:, i, :],
                                                kTs[pb:pb + 64, hp, ts(ki, 128)],
                                                qTs[pb:pb + 64, hp, qsl],
                                                start=True, stop=True)
                                        ex = att.tile([128, 4, BS], BF16, tag="ex",
                                                      name=f"ex{slot}{hp}{h01}{kdone}")
                                        nc.scalar.activation(ex[:, :gsz, :], ps_s[:, :gsz, :], AF.Exp)
                                        if "nomaskmul" not in ablate:
                                            nc.vector.tensor_tensor(
                                                ex[:, :gsz, :], ex[:, :gsz, :],
                                                amask_s[:, base + kdone:base + kdone + gsz, :],
                                                ALU.mult)
                                        for i in range(gsz):
                                            ki = kdone + i
                                            nc.tensor.matmul(
                                                ps_ys[h01][:],
                                                Vs[:, ki, h, :],
                                                ex[:, i, :],
                                                start=(ki == 0), stop=(ki == kc - 1))
                                    kdone += gsz
                                for h01 in (0, 1):
                                    pb = h01 * 64
                                    rr = att.tile([1, BS], F32, tag="rr",
                                                  name=f"rr{slot}{hp}{h01}")
                                    nc.vector.reciprocal(rr[:], ps_ys[h01][64:65, :])
                                    rbs = att.tile([64, BS], F32, tag="rbs",
                                                   name=f"rbs{slot}{hp}{h01}")
                                    nc.gpsimd.partition_broadcast(rbs[:], rr[:])
                                    nc.vector.tensor_tensor(yT[pb:pb + 64, hp, qsl],
                                                            ps_ys[h01][0:64, :],
                                                            rbs[:], ALU.mult)

                # blend: yT = vbar + (yT - vbar) * sel
                vb_b = vbarT[:].to_broadcast([128, EC, R])
                sel_b = selb_s[:, None, :].to_broadcast([128, EC, R])
                nc.vector.tensor_tensor(yT[:], yT[:], vb_b, ALU.subtract)
                nc.vector.tensor_tensor(yT[:], yT[:], sel_b, ALU.mult)
                nc.vector.tensor_tensor(yT[:], yT[:], vb_b, ALU.add)

            # ------- L2c: proj / LN2 / MLP -------
            with tc.tile_pool(name="l2c", bufs=1) as l2c:
                x1T = l2c.tile([128, EC, R], F32)
                z2T = l2c.tile([128, EC, R], BF16)
                hT = l2c.tile([128, FC, R], BF16)
                outT = l2c.tile([128, EC, R], F32)

                if "noproj" in ablate:
                    nc.vector.memset(x1T[:], 0.001)
                for m in range(EC if (stage_limit >= 5 and "noproj" not in ablate) else 0):
                    ps = gpsum.tile([128, 512], F32, tag="gp", name=f"psp{m}")
                    for c in range(HC):
                        nc.tensor.matmul(ps[:, :R], wp_s[:, c, ts(m, 128)], yT[:, c, :],
                                         start=(c == 0), stop=(c == HC - 1))
                    nc.vector.tensor_tensor(x1T[:, m, :], ps[:, :R], xo_s[:, m, :], ALU.add)
                    if flags["bproj"]:
                        nc.vector.tensor_scalar(x1T[:, m, :], x1T[:, m, :],
                                                bp_c[:, m:m + 1], None, ALU.add)

                if "noln2" in ablate:
                    nc.vector.memset(z2T[:], 0.001)
                if stage_limit >= 5 and "noln2" not in ablate:
                    with tc.tile_pool(name="l3c", bufs=2) as l3c, \
                         tc.tile_pool(name="lnab2", bufs=2) as lnab2, \
                         tc.tile_pool(name=f"statp2{ri}", bufs=2, space="PSUM") as statp2:
                        ab = ln_stats_group(l3c, statp2, lnab2, x1T, 0, R, "2")
                        ln_apply_group(x1T, z2T, 0, R, ab, (2, 3))

                do_mlp = (stage_limit >= 6 and "nomlp" not in ablate
                          and "noout" not in ablate)
                with tc.tile_pool(name="wstream", bufs=2) as wstream, \
                     tc.tile_pool(name=f"mlpo{ri}", bufs=1, space="PSUM") as mlpo:
                    if not do_mlp:
                        nc.vector.memset(hT[:], 0.001)
                        nc.vector.memset(outT[:], 0.001)
                    pso = [mlpo.tile([128, 512], F32, tag=f"o{m}", name=f"pso{m}")
                           for m in range(EC)] if do_mlp else []
                    # k-major: silu(z2 @ wfc[k]) immediately feeds the wout
                    # accumulation for all EC output chunks -> no fc/wout barrier
                    for k0 in range(0, FC if do_mlp else 0, WFCG):
                        nk = min(WFCG, FC - k0)
                        wfc_g = wstream.tile([128, WFCG, EC, 128], BF16, tag="wfc",
                                             name=f"wfc{k0}")
                        nc.sync.dma_start(
                            wfc_g[:, :nk], d_wfc[k0:k0 + nk].rearrange(
                                "k p (c n) -> p k c n", c=EC))
                        wout_g = wstream.tile([128, WFCG, EC, 128], BF16, tag="wog",
                                              name=f"wog{k0}")
                        nc.sync.dma_start(
                            wout_g[:, :nk], d_wout[k0:k0 + nk].rearrange(
                                "k p (c n) -> p k c n", c=EC))
                        for ki in range(nk):
                            k = k0 + ki
                            ps = gpsum.tile([128, 512], F32, tag="gp", name=f"psh{k}")
                            for c in range(EC):
                                nc.tensor.matmul(ps[:, :R], wfc_g[:, ki, c, :], z2T[:, c, :],
                                                 start=(c == 0), stop=(c == EC - 1))
                            if flags["bfc"]:
                                nc.scalar.activation(hT[:, k, :], ps[:, :R], AF.Silu,
                                                     bias=bfc_c[:, k:k + 1])
                            else:
                                nc.scalar.activation(hT[:, k, :], ps[:, :R], AF.Silu)
                            for m in range(EC):
                                nc.tensor.matmul(pso[m][:, :R], wout_g[:, ki, m, :],
                                                 hT[:, k, :],
                                                 start=(k == 0), stop=(k == FC - 1))
                    for m in range(EC if do_mlp else 0):
                        nc.vector.tensor_tensor(outT[:, m, :], pso[m][:, :R],
                                                x1T[:, m, :], ALU.add)
                        if flags["bout"]:
                            nc.vector.tensor_scalar(outT[:, m, :], outT[:, m, :],
                                                    bout_c[:, m:m + 1], None, ALU.add)

                nc.sync.dma_start(d_out.rearrange("p (c n) -> p c n", c=EC), outT[:])

        if loop_n > 1:
            with tc.For_i(0, loop_n, 1):
                emit_body(0)
        else:
            for _ri in range(repeat):
                emit_body(_ri)

    nc.finalize()
    return nc


# ----------------------------------------------------------------------------
# host side: input prep / output assembly
# ----------------------------------------------------------------------------

def prepare_in_maps(cfg: Cfg, pairs, jmax, flags, inputs):
    """Build per-core input maps. Returns (in_maps, percore_blocks)."""
    x = np.asarray(inputs["x"], np.float32)
    w_qkv = np.asarray(inputs["w_qkv"], np.float32)
    b_qkv = np.asarray(inputs["b_qkv"], np.float32)
    w_proj = np.asarray(inputs["w_proj"], np.float32)
    b_proj = np.asarray(inputs["b_proj"], np.float32)
    w_fc = np.asarray(inputs["w_fc"], np.float32)
    b_fc = np.asarray(inputs["b_fc"], np.float32)
    w_out = np.asarray(inputs["w_out"], np.float32)
    b_out = np.asarray(inputs["b_out"], np.float32)
    ln1_s = np.asarray(inputs["ln1_scale"], np.float32)
    ln1_b = np.asarray(inputs["ln1_bias"], np.float32)
    ln2_s = np.asarray(inputs["ln2_scale"], np.float32)
    ln2_b = np.asarray(inputs["ln2_bias"], np.float32)
    mask_len = np.asarray(inputs["mask_len"]).astype(np.int64)

    E, L, H, D, BS = cfg.E, cfg.L, cfg.H, cfg.D, cfg.BS
    EC, FC = cfg.EC, cfg.FC
    qscale = 1.0 / math.sqrt(D)

    # split qkv columns: col = h*3D + {0..D-1:q, D..2D-1:k, 2D..3D-1:v}
    wsplit = w_qkv.reshape(E, H, 3 * D)
    wq = np.ascontiguousarray(wsplit[:, :, 0:D].reshape(E, E))
    wk = np.ascontiguousarray(wsplit[:, :, D:2 * D].reshape(E, E))
    wv = np.ascontiguousarray(wsplit[:, :, 2 * D:3 * D].reshape(E, E))
    bsplit = b_qkv.reshape(H, 3 * D)
    bq = np.ascontiguousarray(bsplit[:, 0:D].reshape(E))
    bk = np.ascontiguousarray(bsplit[:, D:2 * D].reshape(E))
    bv = np.ascontiguousarray(bsplit[:, 2 * D:3 * D].reshape(E))

    def chunked_w(w):  # [E, N] -> partition-major [128, EC*N] bf16
        n = w.shape[1]
        return np.ascontiguousarray(
            w.reshape(EC, 128, n).transpose(1, 0, 2).reshape(128, EC * n)).astype(BF16NP)

    def col_f32(v):    # [E or FF] -> [128, C]
        return np.ascontiguousarray(v.reshape(-1, 128).T).astype(np.float32)

    wq_c, wk_c, wv_c, wp_c = (chunked_w(w) for w in (wq, wk, wv, w_proj))
    wfc_c = np.ascontiguousarray(
        w_fc.reshape(EC, 128, FC, 128).transpose(2, 1, 0, 3).reshape(FC, 128, EC * 128)
    ).astype(BF16NP)
    wout_c = np.ascontiguousarray(w_out.reshape(FC, 128, EC * 128)).astype(BF16NP)

    # packed consts [128, CW]: bq*qscale | bk | bproj | bout | bfc | ln(4xEC)
    cst = np.concatenate([
        col_f32(bq) * qscale, col_f32(bk), col_f32(b_proj), col_f32(b_out),
        col_f32(b_fc),
        col_f32(ln1_s), col_f32(ln1_b), col_f32(ln2_s), col_f32(ln2_b),
    ], axis=1)
    assert cst.shape == (128, cst_width(cfg)), cst.shape

    KS = slot_kcs(cfg, pairs, jmax)

    def core_attnmask(blocks):
        """[sum(KS), 128, BS] plain causal masks for this core's blocks."""
        parts = []
        pi = np.arange(128)[:, None]
        qi = np.arange(BS)[None, :]
        for s, j in enumerate(blocks):
            for kchunk in range(KS[s]):
                parts.append((kchunk * 128 + pi) <= (j * BS + qi))
        if not parts:
            return np.zeros((0, 128, BS), BF16NP)
        return np.stack(parts).astype(BF16NP)

    shared = dict(
        wq=wq_c, wk=wk_c, wv=wv_c, wproj=wp_c, wfc=wfc_c, wout=wout_c,
        cst=np.ascontiguousarray(cst), bv=bv.reshape(1, E).astype(BF16NP),
    )

    in_maps = []
    percore_blocks = []
    for c in range(cfg.n_cores):
        b = c // cfg.NPOS
        g = c % cfg.NPOS
        blocks = pairs[g]
        percore_blocks.append((b, blocks))
        xT = x[b].T  # [E, L]
        own_cols = np.concatenate(
            [np.arange(j * BS, (j + 1) * BS) for j in blocks])
        sel = (own_cols < mask_len[b]).astype(BF16NP)
        selb = np.broadcast_to(sel[None, :], (128, cfg.R))
        E_, L_ = xT.shape
        EC_ = E_ // 128
        m = dict(shared)
        m["xT_full"] = np.ascontiguousarray(
            xT.reshape(EC_, 128, L_).transpose(1, 0, 2).reshape(128, EC_ * L_)).astype(BF16NP)
        xo = xT[:, own_cols]
        m["xT_own"] = np.ascontiguousarray(
            xo.reshape(EC_, 128, -1).transpose(1, 0, 2).reshape(128, -1)).astype(np.float32)
        m["selb"] = np.ascontiguousarray(selb)
        m["attnmask"] = core_attnmask(blocks)
        in_maps.append(m)
    return in_maps, percore_blocks


def assemble_output(cfg: Cfg, results, percore_blocks):
    out = np.zeros((cfg.B, cfg.L, cfg.E), np.float32)
    for c, res in enumerate(results):
        b, blocks = percore_blocks[c]
        oT = res["outT"].reshape(128, cfg.EC, cfg.R).transpose(1, 0, 2).reshape(cfg.E, cfg.R)
        for s, j in enumerate(blocks):
            out[b, j * cfg.BS:(j + 1) * cfg.BS, :] = oT[:, s * cfg.BS:(s + 1) * cfg.BS].T
    return out


def make_flags(inputs):
    def nz(name):
        return bool(np.any(np.asarray(inputs[name]) != 0))
    return dict(
        bq=nz("b_qkv"), bk=nz("b_qkv"), bv=nz("b_qkv"),
        bproj=nz("b_proj"), bfc=nz("b_fc"), bout=nz("b_out"),
        ln1aff=bool(np.any(np.asarray(inputs["ln1_scale"]) != 1)
                    or np.any(np.asarray(inputs["ln1_bias"]) != 0)),
        ln2aff=bool(np.any(np.asarray(inputs["ln2_scale"]) != 1)
                    or np.any(np.asarray(inputs["ln2_bias"]) != 0)),
    )


_cached = {}


def kernel(**inputs) -> np.ndarray:
    cfg = Cfg()
    mask_len = np.asarray(inputs["mask_len"]).astype(np.int64)
    pairs, jmax = plan_blocks(cfg, mask_len)
    flags = make_flags(inputs)
    key = (tuple(map(tuple, pairs)), jmax, tuple(sorted(flags.items())))
    if key not in _cached:
        _cached[key] = build_program(cfg, pairs, jmax, flags)
    nc = _cached[key]
    in_maps, percore_blocks = prepare_in_maps(cfg, pairs, jmax, flags, inputs)
    r = run_bass_kernel_spmd(nc, in_maps, core_ids=list(range(cfg.n_cores)))
    return assemble_output(cfg, r.results, percore_blocks)


if __name__ == "__main__":
    pass


# revision 30
# speedup vs baseline: 7.3682x; 7.3682x over previous
"""Trainium2 Bass kernel for nn_AttentionBlock (pre-LN causal attention + SiLU MLP).

8-core SPMD strategy (data-parallel over batch x sequence-parallel over rows):
  - core c handles sample b = c // NPOS, position g = c % NPOS
  - the L rows of a sample are split into NBLK blocks of BS rows; each core owns
    NBPC blocks, paired to balance causal-attention cost (host-chosen pairing)
  - every core computes LN1 + K^T/V for the full sample (replicated), q/proj/MLP
    only for its own rows.
  - the program is branch-free and identical on every core: per-slot attention
    trip counts are the max over cores, and all per-core differences (which
    rows, causal masking) enter via input data (xT_own, selb, attnmask).

All matmul layouts are "transposed" (feature dim on partitions) so no on-device
transposes are needed anywhere; the host feeds x pre-transposed and re-assembles
the transposed output.

Engine balance: PE does all matmuls; ACT does exp/silu/rsqrt + PSUM->SBUF
copies; Pool (GpSimd) does squares, partition broadcasts and attention mask
multiplies; DVE keeps the remaining elementwise.
"""
import math
from contextlib import ExitStack
from dataclasses import dataclass

import ml_dtypes
import numpy as np

import concourse.bass as bass
import concourse.mybir as mybir
import concourse.tile as tile
from concourse import bacc
from concourse.bass import ds, ts
from concourse.bass_utils import run_bass_kernel_spmd

F32 = mybir.dt.float32
BF16 = mybir.dt.bfloat16
FP8 = mybir.dt.float8e4
DR = mybir.MatmulPerfMode.DoubleRow
FP8_WSCALE = 64.0
AF = mybir.ActivationFunctionType
ALU = mybir.AluOpType
BF16NP = ml_dtypes.bfloat16


@dataclass
class Cfg:
    B: int = 2
    L: int = 2048
    E: int = 768
    H: int = 12
    D: int = 64
    FF: int = 3072
    BS: int = 256          # query block rows
    n_cores: int = 8
    eps: float = 1e-6

    @property
    def NPOS(self):
        return self.n_cores // self.B

    @property
    def NBLK(self):
        return self.L // self.BS

    @property
    def NBPC(self):
        return self.NBLK // self.NPOS   # blocks per core

    @property
    def R(self):
        return self.NBPC * self.BS      # own rows per core

    @property
    def EC(self):
        return self.E // 128

    @property
    def FC(self):
        return self.FF // 128

    @property
    def LC(self):
        return self.L // 128

    @property
    def HC(self):
        return self.H // 2              # head-pair chunks (= EC since E = H*D, D=64)


def plan_blocks(cfg: Cfg, mask_lens):
    """Choose jmax (number of attention-active blocks) and block pairing."""
    mmax = int(max(int(m) for m in mask_lens))
    mmax = max(1, min(cfg.L, mmax))
    jmax = (mmax + cfg.BS - 1) // cfg.BS          # blocks [0, jmax) need causal attn
    def cost(j):
        return (j + 1) if j < jmax else 0
    order = sorted(range(cfg.NBLK), key=lambda j: -cost(j))
    pairs = []
    for g in range(cfg.NPOS):
        blocks = []
        for s in range(cfg.NBPC):
            # snake over sorted order: pair heavy with light
            idx = g if s % 2 == 0 else (cfg.NBLK - 1 - g)
            blocks.append(order[idx])
        pairs.append(tuple(blocks))
    return pairs, jmax


def kc_of(cfg: Cfg, j, jmax):
    """number of 128-wide key chunks block j attends to (0 if mask-free)."""
    if j >= jmax:
        return 0
    return (j + 1) * cfg.BS // 128


def slot_kcs(cfg: Cfg, pairs, jmax):
    """Uniform (branch-free) per-slot key-chunk trip counts: the max over
    cores. Cores whose block needs fewer chunks mask the excess to zero via
    the per-core attnmask input; fully-masked blocks compute garbage that the
    sel blend discards (their denominators stay positive: plain causal
    masks are supplied for every assigned block, active or not)."""
    return [max(kc_of(cfg, pairs[g][s], jmax) for g in range(len(pairs)))
            for s in range(cfg.NBPC)]


# packed small-consts layout (columns of the [128, CW] f32 "cst" input):
#   bq [0:EC)  bk [EC:2EC)  bp [2EC:3EC)  bout [3EC:4EC)
#   bfc [4EC : 4EC+FC)   ln (g1,b1,g2,b2) [4EC+FC : 4EC+FC+4EC)
def cst_width(cfg: Cfg):
    return 8 * cfg.EC + cfg.FC


# ----------------------------------------------------------------------------
# program builder
# ----------------------------------------------------------------------------

def build_program(cfg: Cfg, pairs, jmax, flags, bake_g=None, stage_limit=99, repeat=1,
                  loop_n=1, ablate=()):
    """flags: dict with bools: bq, bk, bv, bproj, bfc, bout, ln1aff, ln2aff"""
    E, L, H, D2, FF, BS, R = cfg.E, cfg.L, cfg.H, cfg.D, cfg.FF, cfg.BS, cfg.R
    EC, FC, LC, HC, NBPC = cfg.EC, cfg.FC, cfg.LC, cfg.HC, cfg.NBPC
    KEYS = jmax * BS
    KC = KEYS // 128
    KS = slot_kcs(cfg, pairs, jmax)      # uniform per-slot trip counts
    NK = sum(KS)
    KTC = max(KS) if KS else 0           # key chunks to compute kT for
    qscale = 1.0 / math.sqrt(cfg.D)
    CW = cst_width(cfg)
    WFCG = 6                             # wfc chunks per streamed DMA

    nc = bacc.Bacc(num_devices=cfg.n_cores)

    # ---- dram I/O ----
    d_xTf = nc.dram_tensor("xT_full", [128, EC * L], BF16, kind="ExternalInput")
    d_xTo = nc.dram_tensor("xT_own", [128, EC * R], BF16, kind="ExternalInput")
    d_wq = nc.dram_tensor("wq", [128, EC * E], BF16, kind="ExternalInput")
    d_wk = nc.dram_tensor("wk", [128, EC * E], BF16, kind="ExternalInput")
    d_wv = nc.dram_tensor("wv", [128, EC * E], BF16, kind="ExternalInput")
    d_wp = nc.dram_tensor("wproj", [128, EC * E], BF16, kind="ExternalInput")
    d_wfc = nc.dram_tensor("wfc", [FC, 128, EC * 128], FP8, kind="ExternalInput")
    d_wout = nc.dram_tensor("wout", [FC, 128, EC * 128], FP8, kind="ExternalInput")
    d_cst = nc.dram_tensor("cst", [128, CW], F32, kind="ExternalInput")
    d_bv = nc.dram_tensor("bv", [1, E], BF16, kind="ExternalInput")
    d_selb = nc.dram_tensor("selb", [128, R], BF16, kind="ExternalInput")
    d_amask = nc.dram_tensor("attnmask", [NK, 128, BS], BF16, kind="ExternalInput")
    d_out = nc.dram_tensor("outT", [128, EC * R], F32, kind="ExternalOutput")

    with tile.TileContext(nc) as tc, ExitStack() as st:
        # ------- L0: persistent pools -------
        cpool = st.enter_context(tc.tile_pool(name="consts", bufs=1))

        wq_s = cpool.tile([128, EC, E], BF16)
        wk_s = cpool.tile([128, EC, E], BF16)
        wv_s = cpool.tile([128, EC, E], BF16)
        wp_s = cpool.tile([128, EC, E], BF16)
        xo_s = cpool.tile([128, EC, R], BF16)
        cst_s = cpool.tile([128, CW], F32)
        bv_s = cpool.tile([1, E], BF16)
        selb_s = cpool.tile([128, R], BF16)
        amask_s = cpool.tile([128, NK, BS], BF16)
        ones_col = cpool.tile([128, 1], BF16)
        ones_colf = cpool.tile([128, 1], F32)
        ones_row = cpool.tile([1, 128], BF16)
        ones_11 = cpool.tile([1, 1], BF16)
        eps_11 = cpool.tile([1, 1], F32)
        yT = cpool.tile([128, HC, R], BF16)
        vbarT = cpool.tile([128, EC, 1], F32)
        vrow = cpool.tile([1, E], BF16)

        # packed const column views
        bq_c = cst_s[:, 0:EC]
        bk_c = cst_s[:, EC:2 * EC]
        bp_c = cst_s[:, 2 * EC:3 * EC]
        bout_c = cst_s[:, 3 * EC:4 * EC]
        bfc_c = cst_s[:, 4 * EC:4 * EC + FC]
        ln_c = cst_s[:, 4 * EC + FC:]      # [128, 4*EC] (g1,b1,g2,b2)

        def emit_body(ri):
          with tc.tile_pool(name=f"gpsum{ri}", bufs=2, space="PSUM") as gpsum, \
               tc.tile_pool(name=f"wstream{ri}", bufs=2) as wstream:
            nc.sync.dma_start(xo_s[:], d_xTo.rearrange("p (c n) -> p c n", c=EC))
            nc.sync.dma_start(wq_s[:], d_wq.rearrange("p (c n) -> p c n", c=EC))
            nc.sync.dma_start(wk_s[:], d_wk.rearrange("p (c n) -> p c n", c=EC))
            nc.sync.dma_start(wv_s[:], d_wv.rearrange("p (c n) -> p c n", c=EC))
            nc.sync.dma_start(wp_s[:], d_wp.rearrange("p (c n) -> p c n", c=EC))
            nc.sync.dma_start(cst_s[:], d_cst[:])
            nc.sync.dma_start(bv_s[:], d_bv[:])
            nc.sync.dma_start(selb_s[:], d_selb[:])
            if "nomaskdma" in ablate:
                nc.vector.memset(amask_s[:], 1.0)
            else:
                nc.sync.dma_start(amask_s[:], d_amask.rearrange("k p q -> p k q"))
            nc.vector.memset(ones_col[:], 1.0)
            nc.vector.memset(ones_colf[:], 1.0)
            nc.vector.memset(ones_row[:], 1.0)
            nc.vector.memset(ones_11[:], 1.0)
            nc.vector.memset(eps_11[:], cfg.eps)
            nc.vector.memset(yT[:], 0.0)

            # ============================================================
            # layernorm helpers (transposed layout: tokens on the free axis)
            # stats per 512-column group -> broadcast a/b rows -> apply
            # ============================================================
            def ln_stats_group(pool, statp, abpool, x_bf, cg0, w, tag):
                ps_su = statp.tile([1, 512], F32, tag="su", name=f"su{tag}{cg0}")
                ps_sq = statp.tile([1, 512], F32, tag="sq", name=f"sq{tag}{cg0}")
                ones_in = ones_colf if x_bf.dtype == F32 else ones_col
                for c in range(EC):
                    nc.tensor.matmul(ps_su[:, :w], ones_in[:], x_bf[:, c, cg0:cg0 + w],
                                     start=(c == 0), stop=(c == EC - 1))
                for c in range(EC):
                    sq = pool.tile([128, 512], BF16, tag="lnsq", name=f"sq{tag}{cg0}{c}")
                    nc.vector.tensor_tensor(sq[:, :w], x_bf[:, c, cg0:cg0 + w],
                                            x_bf[:, c, cg0:cg0 + w], ALU.mult)
                    nc.tensor.matmul(ps_sq[:, :w], ones_col[:], sq[:, :w],
                                     start=(c == 0), stop=(c == EC - 1))
                # mu = su/E ; var = sq/E - mu^2 ; a = rsqrt(var+eps) ; b = -mu*a
                mu = pool.tile([1, 512], F32, tag="lnmu", name=f"mu{tag}{cg0}")
                nc.vector.tensor_scalar_mul(mu[:, :w], ps_su[:, :w], 1.0 / E)
                m2 = pool.tile([1, 512], F32, tag="lnm2", name=f"m2{tag}{cg0}")
                nc.vector.tensor_tensor(m2[:, :w], mu[:, :w], mu[:, :w], ALU.mult)
                va = pool.tile([1, 512], F32, tag="lnva", name=f"va{tag}{cg0}")
                nc.vector.scalar_tensor_tensor(va[:, :w], ps_sq[:, :w], 1.0 / E,
                                               m2[:, :w], ALU.mult, ALU.subtract)
                sd = pool.tile([1, 512], F32, tag="lnsd", name=f"sd{tag}{cg0}")
                nc.scalar.activation(sd[:, :w], va[:, :w], AF.Sqrt, bias=eps_11[:])
                arow = pool.tile([1, 512], BF16, tag="lnar", name=f"ar{tag}{cg0}")
                with nc.allow_low_precision(reason="1/std rounded to bf16 as before"):
                    nc.vector.reciprocal(arow[:, :w], sd[:, :w])
                brow = pool.tile([1, 512], BF16, tag="lnbr", name=f"br{tag}{cg0}")
                nc.vector.scalar_tensor_tensor(brow[:, :w], mu[:, :w], -1.0,
                                               arow[:, :w], ALU.mult, ALU.mult)
                ab = abpool.tile([128, 2, 512], BF16, tag="lnab", name=f"ab{tag}{cg0}")
                nc.gpsimd.partition_broadcast(ab[:, 0, :w], arow[:, :w])
                nc.gpsimd.partition_broadcast(ab[:, 1, :w], brow[:, :w])
                return ab

            def ln_apply_group(x_bf, zT_out, cg0, w, ab, gb_idx):
                gi, bi = gb_idx
                affine = flags["ln1aff"] if gb_idx == (0, 1) else flags["ln2aff"]
                for c in range(EC):
                    nc.vector.tensor_tensor(zT_out[:, c, cg0:cg0 + w],
                                            x_bf[:, c, cg0:cg0 + w], ab[:, 0, :w], ALU.mult)
                    nc.vector.tensor_tensor(zT_out[:, c, cg0:cg0 + w],
                                            zT_out[:, c, cg0:cg0 + w], ab[:, 1, :w], ALU.add)
                    if affine:
                        nc.vector.tensor_scalar(zT_out[:, c, cg0:cg0 + w],
                                                zT_out[:, c, cg0:cg0 + w],
                                                ln_c[:, gi * EC + c:gi * EC + c + 1],
                                                ln_c[:, bi * EC + c:bi * EC + c + 1],
                                                ALU.mult, ALU.add)

            # ------- L2: sample-wide tensors (die after attention) -------
            with tc.tile_pool(name="l2", bufs=1) as l2:
                zT = l2.tile([128, EC, L], BF16, tag="zT", name="zT")
                xf_ap = d_xTf.rearrange("p (c n) -> p c n", c=EC)
                for cg0 in range(0, L, 512):
                    nc.sync.dma_start(zT[:, :, cg0:cg0 + 512],
                                      xf_ap[:, :, cg0:cg0 + 512])
                zqT = l2.tile([128, EC, R], BF16, tag="zqT", name="zqT")
                qTs = l2.tile([128, HC, R], BF16, tag="qTs", name="qTs")
                kTs = l2.tile([128, HC, KEYS], BF16, tag="kTs", name="kTs")
                Vs = l2.tile([128, KC, H, 65], BF16, tag="Vs", name="Vs")
                nc.vector.memset(Vs[:, :, :, 64:65], 1.0)
                if "noqkv" in ablate:
                    nc.vector.memset(qTs[:], 0.001)
                    nc.vector.memset(kTs[:], 0.001)
                    nc.vector.memset(Vs[:, :, :, 0:64], 0.001)

                def emit_kT_group(cg0, w):
                    """kT output for key columns [cg0, cg0+w) (all EC head chunks)."""
                    for m in range(EC):
                        ps = gpsum.tile([128, 512], F32, tag="gp", name=f"psk{m}{cg0}")
                        for c in range(EC):
                            nc.tensor.matmul(ps[:, :w], wk_s[:, c, ts(m, 128)],
                                             zT[:, c, cg0:cg0 + w],
                                             start=(c == 0), stop=(c == EC - 1))
                        if flags["bk"]:
                            nc.scalar.activation(kTs[:, m, cg0:cg0 + w], ps[:, :w],
                                                 AF.Identity, bias=bk_c[:, m:m + 1])
                        else:
                            nc.scalar.copy(kTs[:, m, cg0:cg0 + w], ps[:, :w])

                def emit_V_chunk(r):
                    """V rows for key chunk r: [128(keyrow), H, 64] + ones col."""
                    for n0 in range(0, E, 512):
                        w = min(512, E - n0)
                        ps = gpsum.tile([128, 512], F32, tag="gp", name=f"psv{r}{n0}")
                        for c in range(EC):
                            nc.tensor.matmul(ps[:, :w], zT[:, c, ts(r, 128)],
                                             wv_s[:, c, n0:n0 + w],
                                             start=(c == 0),
                                             stop=(c == EC - 1 and not flags["bv"]))
                        if flags["bv"]:
                            nc.tensor.matmul(ps[:, :w], ones_row[:], bv_s[:, n0:n0 + w],
                                             start=False, stop=True)
                        h0 = n0 // 64
                        nh = w // 64
                        nc.scalar.copy(
                            Vs[:, r, h0:h0 + nh, 0:64],
                            ps[:, :w].rearrange("p (h d) -> p h d", d=64))

                # ------- LN1 over full L, interleaved with kT/V per group ----
                do_ln1 = stage_limit >= 1 and "noln1" not in ablate
                do_qkv = stage_limit >= 2 and "noqkv" not in ablate
                with tc.tile_pool(name="l3", bufs=2) as l3, \
                     tc.tile_pool(name="lnab1", bufs=2) as lnab1, \
                     tc.tile_pool(name=f"statp{ri}", bufs=2, space="PSUM") as statp:
                    # own-row LN (straight off the f32 xo input) + q projection:
                    # depends only on the small xo DMA, so it runs while zT loads
                    if do_ln1:
                        ab = ln_stats_group(l3, statp, lnab1, xo_s, 0, R, "o")
                        ln_apply_group(xo_s, zqT, 0, R, ab, (0, 1))
                    else:
                        nc.vector.tensor_copy(zqT[:], xo_s[:])
                    for m in range(EC if do_qkv else 0):
                        ps = gpsum.tile([128, 512], F32, tag="gp", name=f"psq{m}")
                        for c in range(EC):
                            nc.tensor.matmul(ps[:, :R], wq_s[:, c, ts(m, 128)], zqT[:, c, :],
                                             start=(c == 0), stop=(c == EC - 1))
                        # host pre-scales bq by qscale so Identity(in*qscale + bq') works
                        if flags["bq"]:
                            nc.scalar.activation(qTs[:, m, :], ps[:, :R], AF.Identity,
                                                 bias=bq_c[:, m:m + 1], scale=qscale)
                        else:
                            nc.scalar.activation(qTs[:, m, :], ps[:, :R], AF.Copy,
                                                 scale=qscale)
                    for cg0 in range(0, L, 512):
                        if do_ln1:
                            ab = ln_stats_group(l3, statp, lnab1, zT, cg0, 512, "f")
                            ln_apply_group(zT, zT, cg0, 512, ab, (0, 1))
                        if do_qkv and cg0 < KTC * 128:
                            emit_kT_group(cg0, min(512, KTC * 128 - cg0))
                        if do_qkv:
                            for r in range(cg0 // 128, min(KC, cg0 // 128 + 4)):
                                emit_V_chunk(r)

                # vbar = mean over all L rows of V, per head -> vbarT [128, EC, 1]
                if "novbar" in ablate:
                    nc.vector.memset(vbarT[:], 0.001)
                do_vbar = stage_limit >= 4 and "novbar" not in ablate
                HG = 512 // 65            # heads per vbar psum group
                for h0 in range(0, H if do_vbar else 0, HG):
                    nh = min(HG, H - h0)
                    ps = gpsum.tile([1, 512], F32, tag="gp", name=f"vb{h0}")
                    for r in range(KC):
                        nc.tensor.matmul(ps[:, :nh * 65], ones_col[:],
                                         Vs[:, r, h0:h0 + nh, :],
                                         start=(r == 0), stop=(r == KC - 1))
                    nc.vector.tensor_scalar_mul(
                        vrow[:, h0 * 64:(h0 + nh) * 64].rearrange("p (h d) -> p h d", d=64),
                        ps[:, :nh * 65].rearrange("p (h c) -> p h c", c=65)[:, :, 0:64],
                        1.0 / L)
                if KEYS < L and do_vbar:
                    # tail rows [KEYS, L): vbar += (sum of z rows) @ Wv / L
                    zsum = l2.tile([128, EC, 1], F32, tag="zsum", name="zsum")
                    for c in range(EC):
                        nc.vector.tensor_reduce(zsum[:, c, :], zT[:, c, KEYS:L],
                                                mybir.AxisListType.X, ALU.add)
                    zsumb = l2.tile([128, EC, 1], BF16, tag="zsumb", name="zsumb")
                    nc.vector.tensor_scalar_mul(zsumb[:], zsum[:], 1.0 / L)
                for m in range(EC if do_vbar else 0):
                    ps = gpsum.tile([128, 512], F32, tag="gp", name=f"vbt{m}")
                    nc.tensor.matmul(ps[:, 0:1], vrow[:, ts(m, 128)], ones_11[:],
                                     start=True, stop=(KEYS >= L))
                    if KEYS < L:
                        for c in range(EC):
                            nc.tensor.matmul(ps[:, 0:1], wv_s[:, c, ts(m, 128)],
                                             zsumb[:, c, :],
                                             start=False, stop=(c == EC - 1))
                    nc.vector.tensor_copy(vbarT[:, m, :], ps[:, 0:1])
                if KEYS < L and flags["bv"] and do_vbar:
                    # tail bias: vbar += (L-KEYS)/L * bv  (per hd on partitions)
                    bvt = l2.tile([128, EC, 1], BF16, tag="bvt", name="bvt")
                    nc.sync.dma_start(bvt[:], d_bv.rearrange("o (c p) -> p c o", p=128))
                    nc.vector.tensor_scalar(bvt[:], bvt[:], float(L - KEYS) / L, None,
                                            ALU.mult)
                    nc.vector.tensor_tensor(vbarT[:], vbarT[:], bvt[:], ALU.add)

                # ------- attention (branch-free, uniform trip counts) -------
                with (
                    tc.tile_pool(name="att", bufs=3) as att,
                    tc.tile_pool(name="spsum", bufs=2, space="PSUM") as spsum,
                    tc.tile_pool(name="ypsum", bufs=2, space="PSUM") as ypsum,
                ):
                    if stage_limit >= 3 and "noatt" not in ablate:
                        for slot in range(NBPC):
                            kc = KS[slot]
                            if kc == 0:
                                continue
                            base = sum(KS[:slot])
                            qsl = ds(slot * BS, BS)
                            for hp in range(HC):
                                ps_ys = []
                                for h01 in (0, 1):
                                    ps_y = ypsum.tile([65, BS], F32, tag="y",
                                                      name=f"y{slot}{hp}{h01}")
                                    ps_ys.append(ps_y)
                                kdone = 0
                                while kdone < kc:
                                    gsz = min(4, kc - kdone)
                                    for h01 in (0, 1):
                                        h = 2 * hp + h01
                                        pb = h01 * 64
                                        ps_s = spsum.tile([128, 4, BS], F32, tag="s",
                                                          name=f"s{slot}{hp}{h01}{kdone}")
                                        for i in range(gsz):
                                            ki = kdone + i
                                            nc.tensor.matmul(
                                                ps_s[:, i, :],
                                                kTs[pb:pb + 64, hp, ts(ki, 128)],
                                                qTs[pb:pb + 64, hp, qsl],
                                                start=True, stop=True)
                                        ex = att.tile([128, 4, BS], BF16, tag="ex",
                                                      name=f"ex{slot}{hp}{h01}{kdone}")
                                        nc.scalar.activation(ex[:, :gsz, :], ps_s[:, :gsz, :], AF.Exp)
                                        if "nomaskmul" not in ablate:
                                            nc.vector.tensor_tensor(
                                                ex[:, :gsz, :], ex[:, :gsz, :],
                                                amask_s[:, base + kdone:base + kdone + gsz, :],
                                                ALU.mult)
                                        for i in range(gsz):
                                            ki = kdone + i
                                            nc.tensor.matmul(
                                                ps_ys[h01][:],
                                                Vs[:, ki, h, :],
                                                ex[:, i, :],
                                                start=(ki == 0), stop=(ki == kc - 1))
                                    kdone += gsz
                                for h01 in (0, 1):
                                    pb = h01 * 64
                                    rr = att.tile([1, BS], F32, tag="rr",
                                                  name=f"rr{slot}{hp}{h01}")
                                    nc.vector.reciprocal(rr[:], ps_ys[h01][64:65, :])
                                    rbs = att.tile([64, BS], F32, tag="rbs",
                                                   name=f"rbs{slot}{hp}{h01}")
                                    nc.gpsimd.partition_broadcast(rbs[:], rr[:])
                                    nc.vector.tensor_tensor(yT[pb:pb + 64, hp, qsl],
                                                            ps_ys[h01][0:64, :],
                                                            rbs[:], ALU.mult)

                # blend: yT = vbar + (yT - vbar) * sel
                vb_b = vbarT[:].to_broadcast([128, EC, R])
                sel_b = selb_s[:, None, :].to_broadcast([128, EC, R])
                nc.vector.tensor_tensor(yT[:], yT[:], vb_b, ALU.subtract)
                nc.vector.tensor_tensor(yT[:], yT[:], sel_b, ALU.mult)
                nc.vector.tensor_tensor(yT[:], yT[:], vb_b, ALU.add)

            # ------- L2c: proj / LN2 / MLP -------
            with tc.tile_pool(name="l2c", bufs=1) as l2c:
                x1T = l2c.tile([128, EC, R], F32)
                z2T = l2c.tile([128, EC, R], FP8)
                hT = l2c.tile([128, FC, R], FP8)
                outT = l2c.tile([128, EC, R], F32)

                if "noproj" in ablate:
                    nc.vector.memset(x1T[:], 0.001)
                for m in range(EC if (stage_limit >= 5 and "noproj" not in ablate) else 0):
                    ps = gpsum.tile([128, 512], F32, tag="gp", name=f"psp{m}")
                    for c in range(HC):
                        nc.tensor.matmul(ps[:, :R], wp_s[:, c, ts(m, 128)], yT[:, c, :],
                                         start=(c == 0), stop=(c == HC - 1))
                    nc.vector.tensor_tensor(x1T[:, m, :], ps[:, :R], xo_s[:, m, :], ALU.add)
                    if flags["bproj"]:
                        nc.vector.tensor_scalar(x1T[:, m, :], x1T[:, m, :],
                                                bp_c[:, m:m + 1], None, ALU.add)

                if "noln2" in ablate:
                    nc.vector.memset(z2T[:], 0.001)
                if stage_limit >= 5 and "noln2" not in ablate:
                    with tc.tile_pool(name="l3c", bufs=2) as l3c, \
                         tc.tile_pool(name="lnab2", bufs=2) as lnab2, \
                         tc.tile_pool(name=f"statp2{ri}", bufs=2, space="PSUM") as statp2:
                        ab = ln_stats_group(l3c, statp2, lnab2, x1T, 0, R, "2")
                        ln_apply_group(x1T, z2T, 0, R, ab, (2, 3))

                do_mlp = (stage_limit >= 6 and "nomlp" not in ablate
                          and "noout" not in ablate)
                with tc.tile_pool(name=f"mlpo{ri}", bufs=1, space="PSUM") as mlpo:
                    if not do_mlp:
                        nc.vector.memset(hT[:], 0.001)
                        nc.vector.memset(outT[:], 0.001)
                    pso = [mlpo.tile([128, 512], F32, tag=f"o{m}", name=f"pso{m}")
                           for m in range(EC)] if do_mlp else []
                    # k-major: silu(z2 @ wfc[k]) immediately feeds the wout
                    # accumulation for all EC output chunks -> no fc/wout barrier
                    for k0 in range(0, FC if do_mlp else 0, WFCG):
                        nk = min(WFCG, FC - k0)
                        wfc_g = wstream.tile([128, WFCG, EC, 128], FP8, tag="wfc",
                                             name=f"wfc{k0}")
                        nc.sync.dma_start(
                            wfc_g[:, :nk], d_wfc[k0:k0 + nk].rearrange(
                                "k p (c n) -> p k c n", c=EC))
                        wout_g = wstream.tile([128, WFCG, EC, 128], FP8, tag="wog",
                                              name=f"wog{k0}")
                        nc.sync.dma_start(
                            wout_g[:, :nk], d_wout[k0:k0 + nk].rearrange(
                                "k p (c n) -> p k c n", c=EC))
                        for ki in range(nk):
                            k = k0 + ki
                            ps = gpsum.tile([128, 512], F32, tag="gp", name=f"psh{k}")
                            for c2 in range(EC // 2):
                                nc.tensor.matmul(ps[:, :R],
                                                 wfc_g[:, ki, 2 * c2:2 * c2 + 2, :],
                                                 z2T[:, 2 * c2:2 * c2 + 2, :],
                                                 perf_mode=DR,
                                                 start=(c2 == 0), stop=(c2 == EC // 2 - 1))
                            # weights are host-scaled by FP8_WSCALE; descale here
                            if flags["bfc"]:
                                nc.scalar.activation(hT[:, k, :], ps[:, :R], AF.Silu,
                                                     bias=bfc_c[:, k:k + 1],
                                                     scale=1.0 / FP8_WSCALE)
                            else:
                                nc.scalar.activation(hT[:, k, :], ps[:, :R], AF.Silu,
                                                     scale=1.0 / FP8_WSCALE)
                            if k % 2 == 1:
                                for m in range(EC):
                                    nc.tensor.matmul(pso[m][:, :R],
                                                     wout_g[:, ki - 1:ki + 1, m, :],
                                                     hT[:, k - 1:k + 1, :],
                                                     perf_mode=DR,
                                                     start=(k == 1), stop=(k == FC - 1))
                    out_ap = d_out.rearrange("p (c n) -> p c n", c=EC)
                    for m in range(EC if do_mlp else 0):
                        nc.vector.scalar_tensor_tensor(outT[:, m, :], pso[m][:, :R],
                                                       1.0 / FP8_WSCALE,
                                                       x1T[:, m, :], ALU.mult, ALU.add)
                        if flags["bout"]:
                            nc.vector.tensor_scalar(outT[:, m, :], outT[:, m, :],
                                                    bout_c[:, m:m + 1], None, ALU.add)
                        nc.sync.dma_start(out_ap[:, m, :], outT[:, m, :])
                    if not do_mlp:
                        nc.sync.dma_start(out_ap[:], outT[:])

        if loop_n > 1:
            with tc.For_i(0, loop_n, 1):
                emit_body(0)
        else:
            for _ri in range(repeat):
                emit_body(_ri)

    nc.finalize()
    return nc


# ----------------------------------------------------------------------------
# host side: input prep / output assembly
# ----------------------------------------------------------------------------

def prepare_in_maps(cfg: Cfg, pairs, jmax, flags, inputs):
    """Build per-core input maps. Returns (in_maps, percore_blocks)."""
    x = np.asarray(inputs["x"], np.float32)
    w_qkv = np.asarray(inputs["w_qkv"], np.float32)
    b_qkv = np.asarray(inputs["b_qkv"], np.float32)
    w_proj = np.asarray(inputs["w_proj"], np.float32)
    b_proj = np.asarray(inputs["b_proj"], np.float32)
    w_fc = np.asarray(inputs["w_fc"], np.float32)
    b_fc = np.asarray(inputs["b_fc"], np.float32)
    w_out = np.asarray(inputs["w_out"], np.float32)
    b_out = np.asarray(inputs["b_out"], np.float32)
    ln1_s = np.asarray(inputs["ln1_scale"], np.float32)
    ln1_b = np.asarray(inputs["ln1_bias"], np.float32)
    ln2_s = np.asarray(inputs["ln2_scale"], np.float32)
    ln2_b = np.asarray(inputs["ln2_bias"], np.float32)
    mask_len = np.asarray(inputs["mask_len"]).astype(np.int64)

    E, L, H, D, BS = cfg.E, cfg.L, cfg.H, cfg.D, cfg.BS
    EC, FC = cfg.EC, cfg.FC
    qscale = 1.0 / math.sqrt(D)

    # split qkv columns: col = h*3D + {0..D-1:q, D..2D-1:k, 2D..3D-1:v}
    wsplit = w_qkv.reshape(E, H, 3 * D)
    wq = np.ascontiguousarray(wsplit[:, :, 0:D].reshape(E, E))
    wk = np.ascontiguousarray(wsplit[:, :, D:2 * D].reshape(E, E))
    wv = np.ascontiguousarray(wsplit[:, :, 2 * D:3 * D].reshape(E, E))
    bsplit = b_qkv.reshape(H, 3 * D)
    bq = np.ascontiguousarray(bsplit[:, 0:D].reshape(E))
    bk = np.ascontiguousarray(bsplit[:, D:2 * D].reshape(E))
    bv = np.ascontiguousarray(bsplit[:, 2 * D:3 * D].reshape(E))

    def chunked_w(w):  # [E, N] -> partition-major [128, EC*N] bf16
        n = w.shape[1]
        return np.ascontiguousarray(
            w.reshape(EC, 128, n).transpose(1, 0, 2).reshape(128, EC * n)).astype(BF16NP)

    def col_f32(v):    # [E or FF] -> [128, C]
        return np.ascontiguousarray(v.reshape(-1, 128).T).astype(np.float32)

    wq_c, wk_c, wv_c, wp_c = (chunked_w(w) for w in (wq, wk, wv, w_proj))
    FP8NP = ml_dtypes.float8_e4m3fn
    wfc_c = np.ascontiguousarray(
        w_fc.reshape(EC, 128, FC, 128).transpose(2, 1, 0, 3).reshape(FC, 128, EC * 128)
        * 64.0).astype(FP8NP)
    wout_c = np.ascontiguousarray(w_out.reshape(FC, 128, EC * 128) * 64.0).astype(FP8NP)

    # packed consts [128, CW]: bq*qscale | bk | bproj | bout | bfc | ln(4xEC)
    cst = np.concatenate([
        col_f32(bq) * qscale, col_f32(bk), col_f32(b_proj), col_f32(b_out),
        col_f32(b_fc),
        col_f32(ln1_s), col_f32(ln1_b), col_f32(ln2_s), col_f32(ln2_b),
    ], axis=1)
    assert cst.shape == (128, cst_width(cfg)), cst.shape

    KS = slot_kcs(cfg, pairs, jmax)

    def core_attnmask(blocks):
        """[sum(KS), 128, BS] plain causal masks for this core's blocks."""
        parts = []
        pi = np.arange(128)[:, None]
        qi = np.arange(BS)[None, :]
        for s, j in enumerate(blocks):
            for kchunk in range(KS[s]):
                parts.append((kchunk * 128 + pi) <= (j * BS + qi))
        if not parts:
            return np.zeros((0, 128, BS), BF16NP)
        return np.stack(parts).astype(BF16NP)

    shared = dict(
        wq=wq_c, wk=wk_c, wv=wv_c, wproj=wp_c, wfc=wfc_c, wout=wout_c,
        cst=np.ascontiguousarray(cst), bv=bv.reshape(1, E).astype(BF16NP),
    )

    in_maps = []
    percore_blocks = []
    for c in range(cfg.n_cores):
        b = c // cfg.NPOS
        g = c % cfg.NPOS
        blocks = pairs[g]
        percore_blocks.append((b, blocks))
        xT = x[b].T  # [E, L]
        own_cols = np.concatenate(
            [np.arange(j * BS, (j + 1) * BS) for j in blocks])
        sel = (own_cols < mask_len[b]).astype(BF16NP)
        selb = np.broadcast_to(sel[None, :], (128, cfg.R))
        E_, L_ = xT.shape
        EC_ = E_ // 128
        m = dict(shared)
        m["xT_full"] = np.ascontiguousarray(
            xT.reshape(EC_, 128, L_).transpose(1, 0, 2).reshape(128, EC_ * L_)).astype(BF16NP)
        xo = xT[:, own_cols]
        m["xT_own"] = np.ascontiguousarray(
            xo.reshape(EC_, 128, -1).transpose(1, 0, 2).reshape(128, -1)).astype(BF16NP)
        m["selb"] = np.ascontiguousarray(selb)
        m["attnmask"] = core_attnmask(blocks)
        in_maps.append(m)
    return in_maps, percore_blocks


def assemble_output(cfg: Cfg, results, percore_blocks):
    out = np.zeros((cfg.B, cfg.L, cfg.E), np.float32)
    for c, res in enumerate(results):
        b, blocks = percore_blocks[c]
        oT = res["outT"].reshape(128, cfg.EC, cfg.R).transpose(1, 0, 2).reshape(cfg.E, cfg.R)
        for s, j in enumerate(blocks):
            out[b, j * cfg.BS:(j + 1) * cfg.BS, :] = oT[:, s * cfg.BS:(s + 1) * cfg.BS].T
    return out


def make_flags(inputs):
    def nz(name):
        return bool(np.any(np.asarray(inputs[name]) != 0))
    return dict(
        bq=nz("b_qkv"), bk=nz("b_qkv"), bv=nz("b_qkv"),
        bproj=nz("b_proj"), bfc=nz("b_fc"), bout=nz("b_out"),
        ln1aff=bool(np.any(np.asarray(inputs["ln1_scale"]) != 1)
                    or np.any(np.asarray(inputs["ln1_bias"]) != 0)),
        ln2aff=bool(np.any(np.asarray(inputs["ln2_scale"]) != 1)
                    or np.any(np.asarray(inputs["ln2_bias"]) != 0)),
    )


_cached = {}


def kernel(**inputs) -> np.ndarray:
    cfg = Cfg()
    mask_len = np.asarray(inputs["mask_len"]).astype(np.int64)
    pairs, jmax = plan_blocks(cfg, mask_len)
    flags = make_flags(inputs)
    key = (tuple(map(tuple, pairs)), jmax, tuple(sorted(flags.items())))
    if key not in _cached:
        _cached[key] = build_program(cfg, pairs, jmax, flags)
    nc = _cached[key]
    in_maps, percore_blocks = prepare_in_maps(cfg, pairs, jmax, flags, inputs)
    r = run_bass_kernel_spmd(nc, in_maps, core_ids=list(range(cfg.n_cores)))
    return assemble_output(cfg, r.results, percore_blocks)


if __name__ == "__main__":
    pass
